# revision 1
# baseline (speedup 1.0000x reference)
"""Locally-connected 2D conv (unshared weights), VALID, stride 2 — Trainium2 Bass kernel.

Problem (hardcoded):
  x:       (16, 32, 113, 113) f32
  weights: (56, 56, 32, 3, 3, 64) f32   (H_out, W_out, C_in, kh, kw, C_out)
  bias:    (56, 56, 64) f32
  out:     (16, 64, 56, 56) f32
  out[b,o,u,v] = sum_{c,q,r} x[b,c,2u+q,2v+r] * weights[u,v,c,q,r,o] + bias[u,v,o]

Sharding: H_out split across 8 cores (7 output rows each); each core reads only
its 1/8 of the 231MB weight tensor (the dominant traffic).

Host-side repack: weights/bias/x are rearranged with numpy into the exact SBUF
tile layouts so every device DMA moves >=20KB-contiguous runs per partition
(descriptor count is the DMA bottleneck on trn2: ~70ns/desc/engine), with
partition counts divisible by 16 so the HWDGE sprays descriptors across all 16
SDMA engines.

Per-core compute: for each output location (u,v):
  psum(o=64, b=16) += W_chunk[k, r*64+o].T @ X'[k, b]   for r in 0..2
with contraction k = (q,c) on 96 partitions. One PSUM accumulation group spans
a 28-v bank chunk (start on the first matmul, stop on the last; first write
per byte range overwrites, then accumulates). Bias is added by the DVE during
the PSUM->SBUF copy (broadcast over batch). Matmul operands are float32r
(single-pass PE, ~1.8e-4 rel err vs the f32 reference).
"""

import numpy as np

B = 16
C_IN = 32
C_OUT = 64
H_OUT = 56
W_OUT = 56
KK = 3
STRIDE = 2
H_IN = 113

N_CORES = 8
U_PER = H_OUT // N_CORES          # 7 output rows per core
ROWS_IN = (U_PER - 1) * STRIDE + KK  # 15 input rows per core
J_ROWS = ROWS_IN - (KK - 1)       # 13 rows stored per q-shifted copy
VCHUNK = 28                       # output cols per PSUM bank chunk
NCHUNK = W_OUT // VCHUNK          # 2 chunks per u
XFREE = B * J_ROWS * H_IN         # x' tile free size (f32 elems)
WFREE = VCHUNK * KK * C_OUT       # weight chunk free size (5376)
KPART = C_IN * KK                 # 96 contraction partitions (q,c)

_CACHE = {}


def _build():
    import concourse.mybir as mybir
    from concourse import bacc
    from concourse.tile import TileContext

    f32 = mybir.dt.float32
    f32r = mybir.dt.float32r
    nc = bacc.Bacc("TRN2", target_bir_lowering=False, debug=False,
                   num_devices=N_CORES)
    # Host-prepacked tensors (see _pack_core):
    #   xp[p, b*1469 + j*113 + w] = x[b, c, 2u0+q+j, w],  p = q*32+c
    #   wp[u, ch, p, v*192 + r*64 + o] = weights[u0+u, 28ch+v, c, q, r, o]
    #   bp[o, u*56 + v] = bias[u0+u, v, o]
    xp_in = nc.dram_tensor("xp", [KPART, XFREE], f32r,
                           kind="ExternalInput").ap()
    wp_in = nc.dram_tensor("wp", [U_PER, NCHUNK, KPART, WFREE], f32r,
                           kind="ExternalInput").ap()
    bp_in = nc.dram_tensor("bp", [C_OUT, U_PER * W_OUT], f32,
                           kind="ExternalInput").ap()
    y_out = nc.dram_tensor("y", [B, C_OUT, U_PER, W_OUT], f32,
                           kind="ExternalOutput").ap()

    with TileContext(nc) as tc:
        with tc.tile_pool(name="xpool", bufs=1) as xpool, \
             tc.tile_pool(name="wpool", bufs=3) as wpool, \
             tc.tile_pool(name="opool", bufs=1) as opool, \
             tc.tile_pool(name="pspool", bufs=4, space="PSUM") as pspool:

            # x/bias/out ride the ACT HWDGE ring so the weight stream on the
            # SP ring is never stuck behind them (FIFO per ring)
            xt = xpool.tile([KPART, XFREE], f32r)
            nc.scalar.dma_start(out=xt[:], in_=xp_in[:])
            xt3 = xt.rearrange("p (b hw) -> p b hw", b=B)

            bt = xpool.tile([C_OUT, U_PER * W_OUT], f32)
            nc.scalar.dma_start(out=bt[:], in_=bp_in[:])

            # output staging: partition o, free (b, u, v) -> contiguous dest runs
            out_all = opool.tile([C_OUT, B * U_PER * W_OUT], f32)
            oa3 = out_all.rearrange("p (b uv) -> p b uv", b=B)

            for u in range(U_PER):
                for ch in range(NCHUNK):
                    v0 = ch * VCHUNK
                    wt = wpool.tile([KPART, WFREE], f32r)
                    weng = nc.sync if (u * NCHUNK + ch) % 2 == 0 else nc.scalar
                    weng.dma_start(out=wt[:], in_=wp_in[u, ch])
                    wt3 = wt.rearrange("p (v ro) -> p v ro", v=VCHUNK)

                    ps = pspool.tile([C_OUT, VCHUNK * B], f32)
                    for vl in range(VCHUNK):
                        v = v0 + vl
                        for r in range(KK):
                            lhsT = wt3[:, vl:vl + 1,
                                       r * C_OUT:(r + 1) * C_OUT]
                            col = (2 * u) * H_IN + STRIDE * v + r
                            rhs = xt3[:, :, col:col + 1]
                            nc.tensor.matmul(
                                ps[:, vl * B:(vl + 1) * B], lhsT, rhs,
                                start=(vl == 0 and r == 0),
                                stop=(vl == VCHUNK - 1 and r == KK - 1),
                            )
                    ps3 = ps.rearrange("p (v b) -> p b v", v=VCHUNK)
                    uv = u * W_OUT + v0
                    bslice = bt[:, uv:uv + VCHUNK].unsqueeze(1).broadcast_to(
                        [C_OUT, B, VCHUNK])
                    nc.vector.tensor_add(
                        oa3[:, :, uv:uv + VCHUNK], ps3, bslice)

            ydst = y_out.rearrange("b o u v -> o b (u v)")
            nc.scalar.dma_start(out=ydst, in_=out_all.rearrange(
                "p (b uv) -> p b uv", b=B))

    nc.compile()
    return nc


def _get_nc():
    if "nc" not in _CACHE:
        _CACHE["nc"] = _build()
    return _CACHE["nc"]


def _pack_core(x, weights, bias, i):
    u0 = i * U_PER
    # x': (96, B*13*113); p = q*32+c holds x[b, c, 2u0+q+j, w]
    xs = x[:, :, STRIDE * u0:STRIDE * u0 + ROWS_IN, :]      # (B, C, 15, 113)
    xq = np.stack([xs[:, :, q:q + J_ROWS, :] for q in range(KK)], axis=0)
    xq = xq.transpose(0, 2, 1, 3, 4)                        # (q, c, b, j, w)
    xp = np.ascontiguousarray(xq.reshape(KPART, XFREE))

    # w': (U_PER, NCHUNK, 96, 5376); p = q*32+c, free (v, r, o)
    ws = weights[u0:u0 + U_PER].reshape(U_PER, NCHUNK, VCHUNK, C_IN, KK, KK,
                                        C_OUT)
    ws = ws.transpose(0, 1, 4, 3, 2, 5, 6)                  # (u, ch, q, c, v, r, o)
    wp = np.ascontiguousarray(ws.reshape(U_PER, NCHUNK, KPART, WFREE))

    # b': (64, 392): bp[o, u*56+v]
    bp = np.ascontiguousarray(
        bias[u0:u0 + U_PER].reshape(U_PER * W_OUT, C_OUT).T)
    return {"xp": xp, "wp": wp, "bp": bp}


def kernel(x, weights, bias, _trace=False, _tmpdir=None):
    from concourse.bass_utils import run_bass_kernel_spmd

    x = np.ascontiguousarray(x, dtype=np.float32)
    weights = np.ascontiguousarray(weights, dtype=np.float32)
    bias = np.ascontiguousarray(bias, dtype=np.float32)

    nc = _get_nc()
    core_ids = list(range(N_CORES))
    in_maps = [_pack_core(x, weights, bias, i) for i in core_ids]
    res = run_bass_kernel_spmd(nc, in_maps, core_ids, trace=_trace,
                               tmpdir=_tmpdir)
    out = np.concatenate([res.results[i]["y"] for i in core_ids], axis=2)
    if _trace:
        _CACHE["last_result"] = res
    return out



# revision 4
# speedup vs baseline: 2.1073x; 2.1073x over previous
"""Locally-connected 2D conv (unshared weights), VALID, stride 2 — Trainium2 Bass kernel.

Problem (hardcoded):
  x:       (16, 32, 113, 113) f32
  weights: (56, 56, 32, 3, 3, 64) f32   (H_out, W_out, C_in, kh, kw, C_out)
  bias:    (56, 56, 64) f32
  out:     (16, 64, 56, 56) f32
  out[b,o,u,v] = sum_{c,q,r} x[b,c,2u+q,2v+r] * weights[u,v,c,q,r,o] + bias[u,v,o]

Sharding: H_out split across 8 cores (7 output rows each).

Design notes (v2):
- The weight tensor is touched exactly once, so this kernel is HBM-traffic
  bound (~13.6 GB/s per SDMA engine x 16 engines ~= 220 GB/s effective per
  core). Weights and x are cast to bf16 on the host (rel err ~1e-3 vs the
  2e-2 gate), dropping per-core input traffic from 38 MB to ~17 MB.
- Matmul orientation: x window columns are the *stationary* operand
  (LDWEIGHTS of 16 columns ~ 13 ns) and the weight blocks are the *moving*
  operand streaming at 1 col/cycle @ 2.4 GHz. The contraction k = (r, c) on
  96 partitions; accumulation over q happens in PSUM (3 matmuls per output
  location). The reverse orientation (weights stationary) pays 64-col
  LDWEIGHTS at 1.2 GHz per location — 2x the array time.
- x is packed on the host with k=(r,c): partition p = r*32+c holds
  x[b, c, row, 2v+r], i.e. only the 56 strided columns each r-tap actually
  reads (1.49x replication instead of 2.6x for the (q,c) packing).
- PSUM: out partitions = batch (16). Four PE column groups (tile_position
  col base 32g) pack 4 v-blocks into one [128, 448] PSUM bank per (u, half):
  v = 28h + 7g + vl. Drained to SBUF by the scalar+vector engines, DMA'd
  out as f32. Bias is added on the host after the gather (host-side numpy,
  like the input repack).
"""

import numpy as np

B = 16
C_IN = 32
C_OUT = 64
H_OUT = 56
W_OUT = 56
KK = 3
STRIDE = 2
H_IN = 113

N_CORES = 8
U_PER = H_OUT // N_CORES          # 7 output rows per core
ROWS_IN = (U_PER - 1) * STRIDE + KK  # 15 input rows per core
KPART = KK * C_IN                 # 96 contraction partitions (r, c)
G = 4                             # PE column groups
VL = 7                            # v per group per half
NH = 2                            # halves per u
XFREE = ROWS_IN * B * W_OUT       # x tile free elems (row, b, v) = 13440
WFREE_U = NH * G * VL * KK * C_OUT  # weight free elems per u = 10752
OFREE_U = NH * VL * C_OUT         # output free elems per (u, g) = 896

_CACHE = {}


def _build():
    import concourse.mybir as mybir
    from concourse import bacc
    from concourse.tile import TileContext

    f32 = mybir.dt.float32
    bf16 = mybir.dt.bfloat16
    nc = bacc.Bacc("TRN2", target_bir_lowering=False, debug=False,
                   num_devices=N_CORES)
    # Host-prepacked tensors (see _pack_core):
    #   xr[p, (row*16 + b)*56 + v] = x[b, c, 2u0+row, 2v+r],  p = r*32+c
    #   wr[u, p, ((((h*4+g)*7+vl)*3+q)*64 + o] = weights[u0+u, 28h+7g+vl, c, q, r, o]
    #   y[g, u, b, (h*7+vl)*64 + o] = out[b, o, u0+u, 28h+7g+vl] (bias not added)
    xr_in = nc.dram_tensor("xr", [KPART, XFREE], bf16,
                           kind="ExternalInput").ap()
    wr_in = nc.dram_tensor("wr", [U_PER, KPART, WFREE_U], bf16,
                           kind="ExternalInput").ap()
    y_out = nc.dram_tensor("y", [G, U_PER, B, OFREE_U], f32,
                           kind="ExternalOutput").ap()

    with TileContext(nc) as tc:
        with tc.tile_pool(name="xpool", bufs=1) as xpool, \
             tc.tile_pool(name="wpool", bufs=4) as wpool, \
             tc.tile_pool(name="opool", bufs=1) as opool, \
             tc.tile_pool(name="pspool", bufs=4, space="PSUM") as pspool:

            # x in 3 row-chunks (5 input rows each) so u=0 can start after
            # the first ~0.9MB lands; weights stream on the SP ring.
            xt = xpool.tile([KPART, XFREE], bf16)
            chunk = 5 * B * W_OUT
            for ci in range(3):
                nc.scalar.dma_start(out=xt[:, ci * chunk:(ci + 1) * chunk],
                                    in_=xr_in[:, ci * chunk:(ci + 1) * chunk])
            xt4 = xt.rearrange("p (row b v) -> p row b v", row=ROWS_IN, b=B)

            stage = opool.tile([128, U_PER * OFREE_U], f32)
            st3 = stage.rearrange("p (u x) -> p u x", u=U_PER)

            for u in range(U_PER):
                wt = wpool.tile([KPART, WFREE_U], bf16)
                nc.sync.dma_start(out=wt[:], in_=wr_in[u])
                wt6 = wt.rearrange("p (h g vl q o) -> p h g vl q o",
                                   h=NH, g=G, vl=VL, q=KK)

                for h in range(NH):
                    ps = pspool.tile([128, VL * C_OUT], f32)
                    ps3 = ps.rearrange("p (vl o) -> p vl o", vl=VL)
                    # each col group needs its own start=True: the bank clear
                    # only covers the partitions that matmul writes
                    for g in range(G):
                        for vl in range(VL):
                            v = 28 * h + 7 * g + vl
                            for q in range(KK):
                                lhsT = xt4[:, 2 * u + q, :, v]    # [96, 16]
                                rhs = wt6[:, h, g, vl, q]         # [96, 64]
                                nc.tensor.matmul(
                                    ps3[32 * g:32 * g + 16, vl], lhsT, rhs,
                                    start=(vl == 0 and q == 0),
                                    stop=(vl == VL - 1 and q == KK - 1),
                                    tile_position=(0, 32 * g))
                    # whole-tile drain (garbage partitions 16:32 etc. are
                    # copied but never DMA'd); one engine per tile so
                    # scalar/vector never share a PSUM bank
                    dst = st3[:, u, 448 * h:448 * (h + 1)]
                    if (u * NH + h) % 2 == 0:
                        nc.scalar.copy(out=dst, in_=ps[:, :])
                    else:
                        nc.vector.tensor_scalar_add(dst, ps[:, :], 0.0)
                for g in range(G):
                    nc.scalar.dma_start(out=y_out[g, u],
                                        in_=st3[32 * g:32 * g + 16, u])

    nc.compile()
    return nc


def _get_nc():
    if "nc" not in _CACHE:
        _CACHE["nc"] = _build()
    return _CACHE["nc"]


def kernel(x, weights, bias, _trace=False, _tmpdir=None):
    import ml_dtypes
    from concourse.bass_utils import run_bass_kernel_spmd

    bf16 = ml_dtypes.bfloat16
    x = np.asarray(x, dtype=np.float32)
    weights = np.asarray(weights, dtype=np.float32)
    bias = np.asarray(bias, dtype=np.float32)

    # wr: (core, u, p=(r,c), (h,g,vl,q,o))
    wb = weights.astype(bf16).reshape(N_CORES, U_PER, NH, G, VL,
                                      C_IN, KK, KK, C_OUT)
    wr_all = np.ascontiguousarray(
        wb.transpose(0, 1, 7, 5, 2, 3, 4, 6, 8)).reshape(
            N_CORES, U_PER, KPART, WFREE_U)

    xb = x.astype(bf16)
    in_maps = []
    for i in range(N_CORES):
        u0 = i * U_PER
        xs = xb[:, :, STRIDE * u0:STRIDE * u0 + ROWS_IN, :]  # (B, C, 15, 113)
        # (r, c, row, b, v): p = r*32+c holds x[b, c, row, 2v+r]
        xq = np.stack([xs[:, :, :, r::2][:, :, :, :W_OUT] for r in range(KK)],
                      axis=0)                                # (r, B, C, 15, 56)
        xr = np.ascontiguousarray(xq.transpose(0, 2, 3, 1, 4)).reshape(
            KPART, XFREE)
        in_maps.append({"xr": xr, "wr": wr_all[i]})

    nc = _get_nc()
    core_ids = list(range(N_CORES))
    res = run_bass_kernel_spmd(nc, in_maps, core_ids, trace=_trace,
                               tmpdir=_tmpdir)
    # y per core: (G, U_PER, B, (h, vl, o)) -> (b, o, core*7+u, 28h+7g+vl)
    ys = np.stack([res.results[i]["y"] for i in core_ids])
    ys = ys.reshape(N_CORES, G, U_PER, B, NH, VL, C_OUT)
    out = np.ascontiguousarray(
        ys.transpose(3, 6, 0, 2, 4, 1, 5)).reshape(B, C_OUT, H_OUT, W_OUT)
    out += bias.transpose(2, 0, 1)[None]
    if _trace:
        _CACHE["last_result"] = res
    return out


# revision 5
# speedup vs baseline: 2.1404x; 1.0157x over previous
"""Locally-connected 2D conv (unshared weights), VALID, stride 2 — Trainium2 Bass kernel.

Problem (hardcoded):
  x:       (16, 32, 113, 113) f32
  weights: (56, 56, 32, 3, 3, 64) f32   (H_out, W_out, C_in, kh, kw, C_out)
  bias:    (56, 56, 64) f32
  out:     (16, 64, 56, 56) f32
  out[b,o,u,v] = sum_{c,q,r} x[b,c,2u+q,2v+r] * weights[u,v,c,q,r,o] + bias[u,v,o]

Sharding: H_out split across 8 cores (7 output rows each).

Design notes (v3):
- The weight tensor is touched exactly once, so the kernel is pure HBM-traffic
  bound. Per-SDMA-engine throughput measures ~15 GB/s here regardless of
  packet size or queue mix (port shared with the sibling NeuronCore; all 8
  cores stream concurrently), i.e. ~240 GB/s/core. So: minimize bytes, keep
  all 16 engines fed the whole span.
- Weights/x stream as bf16 (rel err ~2.5e-3 vs the 2e-2 gate), output in
  bf16 too: 14.45 (w) + 2.58 (x) + 0.8 (out) MB per core.
- Matmul: x window columns are the *stationary* operand (16-col LDWEIGHTS
  ~13 ns) and weight blocks are the *moving* operand at 1 col/cycle@2.4GHz.
  Contraction k = (r, c) on 96 partitions; q accumulates in PSUM. x is packed
  host-side so partition p = r*32+c holds x[b, c, row, 2v+r] (1.49x
  replication, vs 2.6x for the (q,c) packing).
- PSUM: out partitions = batch (16). Four PE column groups (tile_position
  (0, 32g)) pack v = 28h+7g+vl into one [128, 448] bank per (u, h). Each
  group needs its own start=True (the has_written clear only covers the
  partitions that matmul writes). vl-outer issue order lets the 4 column
  groups run concurrently in the array.
- Weight DMA in 14 per-(u,h) chunks (1MB each) alternating over both HWDGE
  rings, bufs=8 so prefetch never stalls on tile recycling. Bias is added on
  the host after the gather (host-side numpy, like the input repack).
"""

import numpy as np

B = 16
C_IN = 32
C_OUT = 64
H_OUT = 56
W_OUT = 56
KK = 3
STRIDE = 2
H_IN = 113

N_CORES = 8
U_PER = H_OUT // N_CORES          # 7 output rows per core
ROWS_IN = (U_PER - 1) * STRIDE + KK  # 15 input rows per core
KPART = KK * C_IN                 # 96 contraction partitions (r, c)
G = 4                             # PE column groups
VL = 7                            # v per group per half
NH = 2                            # halves per u
XFREE = ROWS_IN * B * W_OUT       # x tile free elems (row, b, v) = 13440
WFREE_H = G * VL * KK * C_OUT     # weight free elems per (u, h) = 5376
OFREE_U = NH * VL * C_OUT         # output free elems per (u, g) = 896

_CACHE = {}


def _build():
    import concourse.mybir as mybir
    from concourse import bacc
    from concourse.tile import TileContext

    bf16 = mybir.dt.bfloat16
    f32 = mybir.dt.float32
    nc = bacc.Bacc("TRN2", target_bir_lowering=False, debug=False,
                   num_devices=N_CORES)
    # Host-prepacked tensors (see kernel()):
    #   xr[p, (row*16 + b)*56 + v] = x[b, c, 2u0+row, 2v+r],  p = r*32+c
    #   wr[u, h, p, (((g*7+vl)*3+q)*64 + o] = weights[u0+u, 28h+7g+vl, c, q, r, o]
    #   y[g, u, b, (h*7+vl)*64 + o] = out[b, o, u0+u, 28h+7g+vl] (no bias)
    xr_in = nc.dram_tensor("xr", [KPART, XFREE], bf16,
                           kind="ExternalInput").ap()
    wr_in = nc.dram_tensor("wr", [U_PER, NH, KPART, WFREE_H], bf16,
                           kind="ExternalInput").ap()
    y_out = nc.dram_tensor("y", [G, U_PER, B, OFREE_U], bf16,
                           kind="ExternalOutput").ap()

    with TileContext(nc) as tc:
        with tc.tile_pool(name="xpool", bufs=1) as xpool, \
             tc.tile_pool(name="wpool", bufs=8) as wpool, \
             tc.tile_pool(name="opool", bufs=1) as opool, \
             tc.tile_pool(name="pspool", bufs=4, space="PSUM") as pspool:

            # x in 3 row-chunks (5 input rows each) so u=0 can start early
            xt = xpool.tile([KPART, XFREE], bf16)
            chunk = 5 * B * W_OUT
            for ci in range(3):
                nc.scalar.dma_start(out=xt[:, ci * chunk:(ci + 1) * chunk],
                                    in_=xr_in[:, ci * chunk:(ci + 1) * chunk])
            xt4 = xt.rearrange("p (row b v) -> p row b v", row=ROWS_IN, b=B)

            stage = opool.tile([128, U_PER * OFREE_U], bf16)
            st3 = stage.rearrange("p (u x) -> p u x", u=U_PER)

            for u in range(U_PER):
                for h in range(NH):
                    wt = wpool.tile([KPART, WFREE_H], bf16)
                    weng = nc.sync if (u * NH + h) % 2 == 0 else nc.scalar
                    weng.dma_start(out=wt[:], in_=wr_in[u, h])
                    wt5 = wt.rearrange("p (g vl q o) -> p g vl q o",
                                       g=G, vl=VL, q=KK)

                    ps = pspool.tile([128, VL * C_OUT], f32)
                    ps3 = ps.rearrange("p (vl o) -> p vl o", vl=VL)
                    # vl-outer: the 4 col groups interleave so they run
                    # concurrently in the array. Each group gets its own
                    # start=True (the bank clear only covers the partitions
                    # that matmul writes).
                    for vl in range(VL):
                        for g in range(G):
                            for q in range(KK):
                                lhsT = xt4[:, 2 * u + q, :, 28 * h + 7 * g + vl]
                                rhs = wt5[:, g, vl, q]            # [96, 64]
                                nc.tensor.matmul(
                                    ps3[32 * g:32 * g + 16, vl], lhsT, rhs,
                                    start=(vl == 0 and q == 0),
                                    stop=(vl == VL - 1 and q == KK - 1),
                                    tile_position=(0, 32 * g))
                    # whole-tile drain on the vector engine (f32->bf16);
                    # garbage partitions 16:32 etc. are copied, never DMA'd
                    nc.vector.tensor_scalar_add(
                        st3[:, u, 448 * h:448 * (h + 1)], ps[:, :], 0.0)
                oeng = nc.scalar if u % 2 == 0 else nc.sync
                for g in range(G):
                    oeng.dma_start(out=y_out[g, u],
                                   in_=st3[32 * g:32 * g + 16, u])

    nc.compile()
    return nc


def _get_nc():
    if "nc" not in _CACHE:
        _CACHE["nc"] = _build()
    return _CACHE["nc"]


def kernel(x, weights, bias, _trace=False, _tmpdir=None):
    import ml_dtypes
    from concourse.bass_utils import run_bass_kernel_spmd

    bf16 = ml_dtypes.bfloat16
    x = np.asarray(x, dtype=np.float32)
    weights = np.asarray(weights, dtype=np.float32)
    bias = np.asarray(bias, dtype=np.float32)

    # wr: (core, u, h, p=(r,c), (g,vl,q,o))
    wb = weights.astype(bf16).reshape(N_CORES, U_PER, NH, G, VL,
                                      C_IN, KK, KK, C_OUT)
    wr_all = np.ascontiguousarray(
        wb.transpose(0, 1, 2, 7, 5, 3, 4, 6, 8)).reshape(
            N_CORES, U_PER, NH, KPART, WFREE_H)

    xb = x.astype(bf16)
    in_maps = []
    for i in range(N_CORES):
        u0 = i * U_PER
        xs = xb[:, :, STRIDE * u0:STRIDE * u0 + ROWS_IN, :]  # (B, C, 15, 113)
        # (r, c, row, b, v): p = r*32+c holds x[b, c, row, 2v+r]
        xq = np.stack([xs[:, :, :, r::2][:, :, :, :W_OUT] for r in range(KK)],
                      axis=0)                                # (r, B, C, 15, 56)
        xr = np.ascontiguousarray(xq.transpose(0, 2, 3, 1, 4)).reshape(
            KPART, XFREE)
        in_maps.append({"xr": xr, "wr": wr_all[i]})

    nc = _get_nc()
    core_ids = list(range(N_CORES))
    res = run_bass_kernel_spmd(nc, in_maps, core_ids, trace=_trace,
                               tmpdir=_tmpdir)
    # y per core: (G, U_PER, B, (h, vl, o)) -> (b, o, core*7+u, 28h+7g+vl)
    ys = np.stack([np.asarray(res.results[i]["y"]) for i in core_ids])
    ys = ys.reshape(N_CORES, G, U_PER, B, NH, VL, C_OUT).astype(np.float32)
    out = np.ascontiguousarray(
        ys.transpose(3, 6, 0, 2, 4, 1, 5)).reshape(B, C_OUT, H_OUT, W_OUT)
    out += bias.transpose(2, 0, 1)[None]
    if _trace:
        _CACHE["last_result"] = res
    return out
